# revision 5
# baseline (speedup 1.0000x reference)
"""Trainium2 Bass kernel for nn_Attention (dense_transformer).

Sharding: 8 cores = 2 batches x 4 heads; each core computes one (batch, head)
attention independently (head/tensor parallel), QKV weights column-sharded and
the output projection row-sharded per head. Host sums the 4 per-head partial
output projections per batch (row-parallel unshard) and adds the bias.

Per-core dataflow (all on-chip, f32):
  x_b [256, 4096] -> q = scale*Wq_h @ x, k = Wk_h @ x          (PE, [64, 4096])
                     vT[m, d] = (x chunkT) @ WvT_h              (PE, [128, 64] blocks)
  T = k^T q   (scores^T, partition = key index m)               (PE)
  E = exp(T)                                                    (ACT, PSUM->SBUF)
  [O; denom] = [v; 1]^T-weights @ E, accumulated over m-blocks  (PE, [65, n])
  U = Wout_h @ O  (unnormalized)                                (PE, [256, n])
Host: out_b = sum_h U_bh / denom_bh + b_out.
"""

from contextlib import ExitStack

import numpy as np

import concourse.bass as bass
import concourse.tile as tile
from concourse import bacc, mybir
from concourse.bass_utils import run_bass_kernel_spmd

HEADS = 4
DIM_HEAD = 64
SCALE = DIM_HEAD**-0.5
B = 2
C = 256  # input channels
N = 4096  # spatial positions (64*64)
NCH = 1024  # n-chunk (query) size of the main pipeline
NB = N // 128  # number of 128-wide key blocks (32)
F32 = mybir.dt.float32

_CACHED_NC = None


def _build_nc() -> bass.Bass:
    """Per-core program; identical on all 8 cores (SPMD), data differs."""
    nc = bacc.Bacc(None, target_bir_lowering=False, debug=False)

    x = nc.declare_dram_parameter("x", [C, N], F32, isOutput=False)
    wq = nc.declare_dram_parameter("wq", [128, 2, DIM_HEAD], F32, isOutput=False)
    wk = nc.declare_dram_parameter("wk", [128, 2, DIM_HEAD], F32, isOutput=False)
    wv = nc.declare_dram_parameter("wv", [128, 2, DIM_HEAD], F32, isOutput=False)
    wo = nc.declare_dram_parameter("wo", [DIM_HEAD, C], F32, isOutput=False)
    u = nc.declare_dram_parameter("u", [C, N], F32, isOutput=True)
    dnm = nc.declare_dram_parameter("dnm", [1, N], F32, isOutput=True)

    with tile.TileContext(nc) as tc, tc.tile_pool(name="singles", bufs=1) as singles:
        x0 = singles.tile([128, N], F32)  # channels 0..127
        x1 = singles.tile([128, N], F32)  # channels 128..255
        wq_sb = singles.tile([128, 2, DIM_HEAD], F32)
        wk_sb = singles.tile([128, 2, DIM_HEAD], F32)
        wv_sb = singles.tile([128, 2, DIM_HEAD], F32)
        wo_sb = singles.tile([DIM_HEAD, C], F32)
        q_sb = singles.tile([DIM_HEAD, N], F32)
        k_sb = singles.tile([DIM_HEAD, N], F32)
        # v'^T blocks: [m-block 128, d 64 + ones column]
        vt_sb = singles.tile([128, NB, DIM_HEAD + 1], F32)

        nc.sync.dma_start(x0[:], x[0:128, :])
        nc.sync.dma_start(x1[:], x[128:256, :])
        nc.sync.dma_start(wq_sb[:], wq[:])
        nc.sync.dma_start(wk_sb[:], wk[:])
        nc.sync.dma_start(wv_sb[:], wv[:])
        nc.sync.dma_start(wo_sb[:], wo[:])
        nc.vector.memset(vt_sb[:, :, DIM_HEAD], 1.0)

        # ---- Phase B: projections ----
        with tc.tile_pool(name="psumB", bufs=2, space="PSUM") as psumB:
            for ch in range(N // 512):
                sl = slice(ch * 512, (ch + 1) * 512)
                ps_q = psumB.tile([DIM_HEAD, 512], F32, tag="q")
                nc.tensor.matmul(ps_q[:], wq_sb[:, 0, :], x0[:, sl], start=True, stop=False)
                nc.tensor.matmul(ps_q[:], wq_sb[:, 1, :], x1[:, sl], start=False, stop=True)
                nc.vector.tensor_copy(q_sb[:, sl], ps_q[:])
                ps_k = psumB.tile([DIM_HEAD, 512], F32, tag="k")
                nc.tensor.matmul(ps_k[:], wk_sb[:, 0, :], x0[:, sl], start=True, stop=False)
                nc.tensor.matmul(ps_k[:], wk_sb[:, 1, :], x1[:, sl], start=False, stop=True)
                nc.vector.tensor_copy(k_sb[:, sl], ps_k[:])
            for mb in range(NB):
                sl = slice(mb * 128, (mb + 1) * 128)
                ps_v = psumB.tile([128, DIM_HEAD], F32, tag="v")
                nc.tensor.matmul(ps_v[:], x0[:, sl], wv_sb[:, 0, :], start=True, stop=False)
                nc.tensor.matmul(ps_v[:], x1[:, sl], wv_sb[:, 1, :], start=False, stop=True)
                nc.vector.tensor_copy(vt_sb[:, mb, 0:DIM_HEAD], ps_v[:])

        # ---- Phase C: attention + output projection, n-chunks of NCH ----
        with (
            tc.tile_pool(name="psumT", bufs=2, space="PSUM") as psumT,
            tc.tile_pool(name="psumO", bufs=1, space="PSUM") as psumO,
            tc.tile_pool(name="psumU", bufs=1, space="PSUM") as psumU,
            tc.tile_pool(name="esb", bufs=3) as esb,
            tc.tile_pool(name="osb", bufs=2) as osb,
            tc.tile_pool(name="usb", bufs=4) as usb,
        ):
            for ci in range(N // NCH):
                n0 = ci * NCH
                ps_o = psumO.tile([DIM_HEAD + 1, NCH], F32)
                for mb in range(NB):
                    msl = slice(mb * 128, (mb + 1) * 128)
                    ps_t = psumT.tile([128, NCH], F32)
                    e_t = esb.tile([128, NCH], F32)
                    for s in range(NCH // 512):
                        ssl = slice(s * 512, (s + 1) * 512)
                        nc.tensor.matmul(
                            ps_t[:, ssl],
                            k_sb[:, msl],
                            q_sb[:, n0 + s * 512 : n0 + (s + 1) * 512],
                            start=True,
                            stop=True,
                        )
                    nc.scalar.activation(e_t[:], ps_t[:], mybir.ActivationFunctionType.Exp)
                    for s in range(NCH // 512):
                        ssl = slice(s * 512, (s + 1) * 512)
                        nc.tensor.matmul(
                            ps_o[:, ssl],
                            vt_sb[:, mb, :],
                            e_t[:, ssl],
                            start=(mb == 0),
                            stop=(mb == NB - 1),
                        )
                o_t = osb.tile([DIM_HEAD + 1, NCH], F32)
                nc.vector.tensor_copy(o_t[:], ps_o[:])
                nc.sync.dma_start(dnm[0:1, n0 : n0 + NCH], o_t[DIM_HEAD : DIM_HEAD + 1, :])
                for half in range(2):
                    osl = slice(half * 128, (half + 1) * 128)
                    ps_u = psumU.tile([128, NCH], F32)
                    for s in range(NCH // 512):
                        ssl = slice(s * 512, (s + 1) * 512)
                        nc.tensor.matmul(
                            ps_u[:, ssl],
                            wo_sb[:, osl],
                            o_t[0:DIM_HEAD, ssl],
                            start=True,
                            stop=True,
                        )
                    u_t = usb.tile([128, NCH], F32)
                    nc.vector.tensor_copy(u_t[:], ps_u[:])
                    nc.sync.dma_start(u[osl, n0 : n0 + NCH], u_t[:])

    nc.compile()
    return nc


def _get_nc() -> bass.Bass:
    global _CACHED_NC
    if _CACHED_NC is None:
        _CACHED_NC = _build_nc()
    return _CACHED_NC


def _stripe_kxm(w: np.ndarray) -> np.ndarray:
    """[256, M] -> [128, 2, M] k-subtile layout (c = t*128 + p)."""
    return np.ascontiguousarray(w.reshape(2, 128, -1).transpose(1, 0, 2))


def make_in_maps(x, w_qkv, w_out):
    x2 = np.ascontiguousarray(x.reshape(B, C, N), dtype=np.float32)
    in_maps = []
    for core in range(8):
        b, h = divmod(core, HEADS)
        hs = slice(h * DIM_HEAD, (h + 1) * DIM_HEAD)
        wq = (w_qkv[0 * C :][hs, :] * SCALE).T  # [256, 64], scale folded
        wk = w_qkv[1 * C :][hs, :].T
        wv = w_qkv[2 * C :][hs, :].T
        wo = w_out[:, hs].T  # [64, 256]
        in_maps.append(
            {
                "x": x2[b],
                "wq": _stripe_kxm(wq.astype(np.float32)),
                "wk": _stripe_kxm(wk.astype(np.float32)),
                "wv": _stripe_kxm(wv.astype(np.float32)),
                "wo": np.ascontiguousarray(wo, dtype=np.float32),
            }
        )
    return in_maps


def combine(results, b_out):
    out = np.zeros((B, C, N), dtype=np.float32)
    for core in range(8):
        b, _h = divmod(core, HEADS)
        r = results[core]
        out[b] += r["u"].reshape(C, N) / r["dnm"].reshape(1, N)
    out += b_out.astype(np.float32)[None, :, None]
    return out.reshape(B, C, 64, 64)


def kernel(x, w_qkv, w_out, b_out, _run_kwargs=None):
    nc = _get_nc()
    in_maps = make_in_maps(np.asarray(x), np.asarray(w_qkv), np.asarray(w_out))
    kw = _run_kwargs or {}
    res = run_bass_kernel_spmd(nc, in_maps, list(range(8)), **kw)
    out = combine(res.results, np.asarray(b_out))
    kernel.last_result = res
    return out


# revision 9
# speedup vs baseline: 1.5979x; 1.5979x over previous
"""Trainium2 Bass kernel for nn_Attention (dense_transformer).

Sharding: 8 cores = 2 batches x 4 heads; each core computes one (batch, head)
attention independently (head/tensor parallel), QKV weights column-sharded and
the output projection row-sharded per head. Host sums the 4 per-head partial
output projections per batch (row-parallel unshard) and adds the bias.

Per-core dataflow (all on-chip, f32):
  x_b [256, 4096] -> q = scale*Wq_h @ x, k = Wk_h @ x          (PE, [64, 4096])
                     vT[m, d] = (x chunkT) @ WvT_h              (PE, [128, 64] blocks)
  T = k^T q   (scores^T, partition = key index m)               (PE)
  E = exp(T)                                                    (ACT, PSUM->SBUF)
  [O; denom] = [v; 1]^T-weights @ E, accumulated over m-blocks  (PE, [65, n])
  U = Wout_h @ O  (unnormalized)                                (PE, [256, n])
Host: out_b = sum_h U_bh / denom_bh + b_out.
"""

from contextlib import ExitStack

import numpy as np

import concourse.bass as bass
import concourse.tile as tile
from concourse import bacc, mybir
from concourse.bass_utils import run_bass_kernel_spmd

HEADS = 4
DIM_HEAD = 64
SCALE = DIM_HEAD**-0.5
B = 2
C = 256  # input channels
N = 4096  # spatial positions (64*64)
NCH = 1024  # n-chunk (query) size of the main pipeline
NB = N // 128  # number of 128-wide key blocks (32)
F32 = mybir.dt.float32
F32R = mybir.dt.float32r  # single-pass PE fp32 (1 cyc/row at N>=256 vs 4 for fp32)


def _r(ap):
    """Bitcast an f32 AP to float32r for PE matmul operands."""
    return ap.bitcast(F32R)

_CACHED_NC = None


def _build_nc() -> bass.Bass:
    """Per-core program; identical on all 8 cores (SPMD), data differs."""
    nc = bacc.Bacc(None, target_bir_lowering=False, debug=False)

    x = nc.declare_dram_parameter("x", [C, N], F32, isOutput=False)
    wq = nc.declare_dram_parameter("wq", [128, 2, DIM_HEAD], F32, isOutput=False)
    wk = nc.declare_dram_parameter("wk", [128, 2, DIM_HEAD], F32, isOutput=False)
    wv = nc.declare_dram_parameter("wv", [128, 2, DIM_HEAD], F32, isOutput=False)
    wo = nc.declare_dram_parameter("wo", [DIM_HEAD, C], F32, isOutput=False)
    u = nc.declare_dram_parameter("u", [C, N], F32, isOutput=True)
    dnm = nc.declare_dram_parameter("dnm", [1, N], F32, isOutput=True)

    with tile.TileContext(nc) as tc, tc.tile_pool(name="singles", bufs=1) as singles:
        x0 = singles.tile([128, N], F32R)  # channels 0..127
        x1 = singles.tile([128, N], F32R)  # channels 128..255
        wq_sb = singles.tile([128, 2, DIM_HEAD], F32R)
        wk_sb = singles.tile([128, 2, DIM_HEAD], F32R)
        wv_sb = singles.tile([128, 2, DIM_HEAD], F32R)
        wo_sb = singles.tile([DIM_HEAD, C], F32R)
        q_sb = singles.tile([DIM_HEAD, N], F32R)
        k_sb = singles.tile([DIM_HEAD, N], F32R)
        # v'^T blocks: [m-block 128, d 64 + ones column]
        vt_sb = singles.tile([128, NB, DIM_HEAD + 1], F32R)

        nc.sync.dma_start(x0[:], x[0:128, :].bitcast(F32R))
        nc.sync.dma_start(x1[:], x[128:256, :].bitcast(F32R))
        nc.sync.dma_start(wq_sb[:], wq[:].bitcast(F32R))
        nc.sync.dma_start(wk_sb[:], wk[:].bitcast(F32R))
        nc.sync.dma_start(wv_sb[:], wv[:].bitcast(F32R))
        nc.sync.dma_start(wo_sb[:], wo[:].bitcast(F32R))
        ones_t = singles.tile([128, 1], F32)
        nc.vector.memset(ones_t[:], 1.0)
        nc.vector.tensor_copy(
            vt_sb[:, :, DIM_HEAD], ones_t[:, 0:1].to_broadcast((128, NB))
        )

        # ---- Phase B: projections ----
        with tc.tile_pool(name="psumB", bufs=2, space="PSUM") as psumB:
            for ch in range(N // 512):
                sl = slice(ch * 512, (ch + 1) * 512)
                ps_q = psumB.tile([DIM_HEAD, 512], F32, tag="q")
                nc.tensor.matmul(ps_q[:], wq_sb[:, 0, :], x0[:, sl], start=True, stop=False)
                nc.tensor.matmul(ps_q[:], wq_sb[:, 1, :], x1[:, sl], start=False, stop=True)
                nc.vector.tensor_copy(q_sb[:, sl], ps_q[:])
                ps_k = psumB.tile([DIM_HEAD, 512], F32, tag="k")
                nc.tensor.matmul(ps_k[:], wk_sb[:, 0, :], x0[:, sl], start=True, stop=False)
                nc.tensor.matmul(ps_k[:], wk_sb[:, 1, :], x1[:, sl], start=False, stop=True)
                nc.vector.tensor_copy(k_sb[:, sl], ps_k[:])
            for mb in range(NB):
                sl = slice(mb * 128, (mb + 1) * 128)
                ps_v = psumB.tile([128, DIM_HEAD], F32, tag="v")
                nc.tensor.matmul(ps_v[:], x0[:, sl], wv_sb[:, 0, :], start=True, stop=False)
                nc.tensor.matmul(ps_v[:], x1[:, sl], wv_sb[:, 1, :], start=False, stop=True)
                nc.vector.tensor_copy(vt_sb[:, mb, 0:DIM_HEAD], ps_v[:])

        # ---- Phase C: attention + output projection, n-chunks of NCH ----
        with (
            tc.tile_pool(name="psumT", bufs=2, space="PSUM") as psumT,
            tc.tile_pool(name="psumO", bufs=1, space="PSUM") as psumO,
            tc.tile_pool(name="psumU", bufs=1, space="PSUM") as psumU,
            tc.tile_pool(name="esb", bufs=3) as esb,
            tc.tile_pool(name="osb", bufs=2) as osb,
            tc.tile_pool(name="usb", bufs=4) as usb,
        ):
            for ci in range(N // NCH):
                n0 = ci * NCH
                ps_o = psumO.tile([DIM_HEAD + 1, NCH], F32)
                for mb in range(NB):
                    msl = slice(mb * 128, (mb + 1) * 128)
                    ps_t = psumT.tile([128, NCH], F32)
                    e_t = esb.tile([128, NCH], F32R)
                    for s in range(NCH // 512):
                        ssl = slice(s * 512, (s + 1) * 512)
                        nc.tensor.matmul(
                            ps_t[:, ssl],
                            k_sb[:, msl],
                            q_sb[:, n0 + s * 512 : n0 + (s + 1) * 512],
                            start=True,
                            stop=True,
                        )
                    nc.scalar.activation(e_t[:], ps_t[:], mybir.ActivationFunctionType.Exp)
                    for s in range(NCH // 512):
                        ssl = slice(s * 512, (s + 1) * 512)
                        nc.tensor.matmul(
                            ps_o[:, ssl],
                            vt_sb[:, mb, :],
                            e_t[:, ssl],
                            start=(mb == 0),
                            stop=(mb == NB - 1),
                        )
                o_t = osb.tile([DIM_HEAD + 1, NCH], F32R)
                nc.vector.tensor_copy(o_t[:], ps_o[:])
                nc.sync.dma_start(dnm[0:1, n0 : n0 + NCH], o_t[DIM_HEAD : DIM_HEAD + 1, :].bitcast(F32))
                for half in range(2):
                    osl = slice(half * 128, (half + 1) * 128)
                    ps_u = psumU.tile([128, NCH], F32)
                    for s in range(NCH // 512):
                        ssl = slice(s * 512, (s + 1) * 512)
                        nc.tensor.matmul(
                            ps_u[:, ssl],
                            wo_sb[:, osl],
                            o_t[0:DIM_HEAD, ssl],
                            start=True,
                            stop=True,
                        )
                    u_t = usb.tile([128, NCH], F32)
                    nc.vector.tensor_copy(u_t[:], ps_u[:])
                    nc.sync.dma_start(u[osl, n0 : n0 + NCH], u_t[:])

    nc.compile()
    return nc


def _get_nc() -> bass.Bass:
    global _CACHED_NC
    if _CACHED_NC is None:
        _CACHED_NC = _build_nc()
    return _CACHED_NC


def _stripe_kxm(w: np.ndarray) -> np.ndarray:
    """[256, M] -> [128, 2, M] k-subtile layout (c = t*128 + p)."""
    return np.ascontiguousarray(w.reshape(2, 128, -1).transpose(1, 0, 2))


def make_in_maps(x, w_qkv, w_out):
    x2 = np.ascontiguousarray(x.reshape(B, C, N), dtype=np.float32)
    in_maps = []
    for core in range(8):
        b, h = divmod(core, HEADS)
        hs = slice(h * DIM_HEAD, (h + 1) * DIM_HEAD)
        wq = (w_qkv[0 * C :][hs, :] * SCALE).T  # [256, 64], scale folded
        wk = w_qkv[1 * C :][hs, :].T
        wv = w_qkv[2 * C :][hs, :].T
        wo = w_out[:, hs].T  # [64, 256]
        in_maps.append(
            {
                "x": x2[b],
                "wq": _stripe_kxm(wq.astype(np.float32)),
                "wk": _stripe_kxm(wk.astype(np.float32)),
                "wv": _stripe_kxm(wv.astype(np.float32)),
                "wo": np.ascontiguousarray(wo, dtype=np.float32),
            }
        )
    return in_maps


def combine(results, b_out):
    out = np.zeros((B, C, N), dtype=np.float32)
    for core in range(8):
        b, _h = divmod(core, HEADS)
        r = results[core]
        out[b] += r["u"].reshape(C, N) / r["dnm"].reshape(1, N)
    out += b_out.astype(np.float32)[None, :, None]
    return out.reshape(B, C, 64, 64)


def kernel(x, w_qkv, w_out, b_out, _run_kwargs=None):
    nc = _get_nc()
    in_maps = make_in_maps(np.asarray(x), np.asarray(w_qkv), np.asarray(w_out))
    kw = _run_kwargs or {}
    res = run_bass_kernel_spmd(nc, in_maps, list(range(8)), **kw)
    out = combine(res.results, np.asarray(b_out))
    kernel.last_result = res
    return out


# revision 10
# speedup vs baseline: 2.5707x; 1.6088x over previous
"""Trainium2 Bass kernel for nn_Attention (dense_transformer).

Sharding: 8 cores = 2 batches x 4 heads; each core computes one (batch, head)
attention independently (head/tensor parallel), QKV weights column-sharded and
the output projection row-sharded per head. Host sums the 4 per-head partial
output projections per batch (row-parallel unshard) and adds the bias.

Per-core dataflow (all on-chip, f32):
  x_b [256, 4096] -> q = scale*Wq_h @ x, k = Wk_h @ x          (PE, [64, 4096])
                     vT[m, d] = (x chunkT) @ WvT_h              (PE, [128, 64] blocks)
  T = k^T q   (scores^T, partition = key index m)               (PE)
  E = exp(T)                                                    (ACT, PSUM->SBUF)
  [O; denom] = [v; 1]^T-weights @ E, accumulated over m-blocks  (PE, [65, n])
  U = Wout_h @ O  (unnormalized)                                (PE, [256, n])
Host: out_b = sum_h U_bh / denom_bh + b_out.
"""

from contextlib import ExitStack

import numpy as np

import concourse.bass as bass
import concourse.tile as tile
from concourse import bacc, mybir
from concourse.bass_utils import run_bass_kernel_spmd

HEADS = 4
DIM_HEAD = 64
SCALE = DIM_HEAD**-0.5
B = 2
C = 256  # input channels
N = 4096  # spatial positions (64*64)
NCH = 1024  # n-chunk (query) size of the main pipeline
NB = N // 128  # number of 128-wide key blocks (32)
F32 = mybir.dt.float32
F32R = mybir.dt.float32r  # single-pass PE fp32 (1 cyc/row at N>=256 vs 4 for fp32)
F16 = mybir.dt.float16  # fast-weight-load matmul path; ~2^-11 operand rounding


def _r(ap):
    """Bitcast an f32 AP to float32r for PE matmul operands."""
    return ap.bitcast(F32R)

_CACHED_NC = None


def _build_nc() -> bass.Bass:
    """Per-core program; identical on all 8 cores (SPMD), data differs."""
    nc = bacc.Bacc(None, target_bir_lowering=False, debug=False)

    x = nc.declare_dram_parameter("x", [C, N], F32, isOutput=False)
    wq = nc.declare_dram_parameter("wq", [128, 2, DIM_HEAD], F32, isOutput=False)
    wk = nc.declare_dram_parameter("wk", [128, 2, DIM_HEAD], F32, isOutput=False)
    wv = nc.declare_dram_parameter("wv", [128, 2, DIM_HEAD], F32, isOutput=False)
    wo = nc.declare_dram_parameter("wo", [DIM_HEAD, C], F32, isOutput=False)
    u = nc.declare_dram_parameter("u", [C, N], F32, isOutput=True)
    dnm = nc.declare_dram_parameter("dnm", [1, N], F32, isOutput=True)

    with tile.TileContext(nc) as tc, tc.tile_pool(name="singles", bufs=1) as singles:
        x0 = singles.tile([128, N], F32R)  # channels 0..127
        x1 = singles.tile([128, N], F32R)  # channels 128..255
        wq_sb = singles.tile([128, 2, DIM_HEAD], F32R)
        wk_sb = singles.tile([128, 2, DIM_HEAD], F32R)
        wv_sb = singles.tile([128, 2, DIM_HEAD], F32R)
        wo_sb = singles.tile([DIM_HEAD, C], F32R)
        q_sb = singles.tile([128, N], F16)
        k_sb = singles.tile([128, N], F16)
        # v'^T blocks: [m-block 128, d 64 + ones column]
        vt_sb = singles.tile([128, NB, DIM_HEAD + 1], F16)

        nc.sync.dma_start(x0[:], x[0:128, :].bitcast(F32R))
        nc.sync.dma_start(x1[:], x[128:256, :].bitcast(F32R))
        nc.sync.dma_start(wq_sb[:], wq[:].bitcast(F32R))
        nc.sync.dma_start(wk_sb[:], wk[:].bitcast(F32R))
        nc.sync.dma_start(wv_sb[:], wv[:].bitcast(F32R))
        nc.sync.dma_start(wo_sb[:], wo[:].bitcast(F32R))
        nc.vector.memset(q_sb[DIM_HEAD:128, :], 0.0)
        nc.vector.memset(k_sb[DIM_HEAD:128, :], 0.0)
        ones_t = singles.tile([128, 1], F32)
        nc.vector.memset(ones_t[:], 1.0)
        nc.vector.tensor_copy(
            vt_sb[:, :, DIM_HEAD], ones_t[:, 0:1].to_broadcast((128, NB))
        )

        # ---- Phase B: projections ----
        with tc.tile_pool(name="psumB", bufs=2, space="PSUM") as psumB:
            for ch in range(N // 512):
                sl = slice(ch * 512, (ch + 1) * 512)
                ps_q = psumB.tile([DIM_HEAD, 512], F32, tag="q")
                nc.tensor.matmul(ps_q[:], wq_sb[:, 0, :], x0[:, sl], start=True, stop=False)
                nc.tensor.matmul(ps_q[:], wq_sb[:, 1, :], x1[:, sl], start=False, stop=True)
                nc.vector.tensor_copy(q_sb[0:DIM_HEAD, sl], ps_q[:])
                ps_k = psumB.tile([DIM_HEAD, 512], F32, tag="k")
                nc.tensor.matmul(ps_k[:], wk_sb[:, 0, :], x0[:, sl], start=True, stop=False)
                nc.tensor.matmul(ps_k[:], wk_sb[:, 1, :], x1[:, sl], start=False, stop=True)
                nc.vector.tensor_copy(k_sb[0:DIM_HEAD, sl], ps_k[:])
            for mb in range(NB):
                sl = slice(mb * 128, (mb + 1) * 128)
                ps_v = psumB.tile([128, DIM_HEAD], F32, tag="v")
                nc.tensor.matmul(ps_v[:], x0[:, sl], wv_sb[:, 0, :], start=True, stop=False)
                nc.tensor.matmul(ps_v[:], x1[:, sl], wv_sb[:, 1, :], start=False, stop=True)
                nc.vector.tensor_copy(vt_sb[:, mb, 0:DIM_HEAD], ps_v[:])

        # ---- Phase C: attention + output projection, n-chunks of NCH ----
        with (
            tc.tile_pool(name="psumT", bufs=2, space="PSUM") as psumT,
            tc.tile_pool(name="psumO", bufs=1, space="PSUM") as psumO,
            tc.tile_pool(name="psumU", bufs=1, space="PSUM") as psumU,
            tc.tile_pool(name="esb", bufs=3) as esb,
            tc.tile_pool(name="osb", bufs=2) as osb,
            tc.tile_pool(name="usb", bufs=4) as usb,
        ):
            for ci in range(N // NCH):
                n0 = ci * NCH
                ps_o = psumO.tile([DIM_HEAD + 1, NCH], F32)
                for mb in range(NB):
                    msl = slice(mb * 128, (mb + 1) * 128)
                    ps_t = psumT.tile([128, NCH], F32)
                    e_t = esb.tile([128, NCH], F16)
                    for s in range(NCH // 512):
                        ssl = slice(s * 512, (s + 1) * 512)
                        nc.tensor.matmul(
                            ps_t[:, ssl],
                            k_sb[:, msl],
                            q_sb[:, n0 + s * 512 : n0 + (s + 1) * 512],
                            start=True,
                            stop=True,
                        )
                    nc.scalar.activation(e_t[:], ps_t[:], mybir.ActivationFunctionType.Exp)
                    for s in range(NCH // 512):
                        ssl = slice(s * 512, (s + 1) * 512)
                        nc.tensor.matmul(
                            ps_o[:, ssl],
                            vt_sb[:, mb, :],
                            e_t[:, ssl],
                            start=(mb == 0),
                            stop=(mb == NB - 1),
                        )
                o_t = osb.tile([DIM_HEAD + 1, NCH], F32R)
                nc.vector.tensor_copy(o_t[:], ps_o[:])
                nc.sync.dma_start(dnm[0:1, n0 : n0 + NCH], o_t[DIM_HEAD : DIM_HEAD + 1, :].bitcast(F32))
                for half in range(2):
                    osl = slice(half * 128, (half + 1) * 128)
                    ps_u = psumU.tile([128, NCH], F32)
                    for s in range(NCH // 512):
                        ssl = slice(s * 512, (s + 1) * 512)
                        nc.tensor.matmul(
                            ps_u[:, ssl],
                            wo_sb[:, osl],
                            o_t[0:DIM_HEAD, ssl],
                            start=True,
                            stop=True,
                        )
                    u_t = usb.tile([128, NCH], F32)
                    nc.vector.tensor_copy(u_t[:], ps_u[:])
                    nc.sync.dma_start(u[osl, n0 : n0 + NCH], u_t[:])

    nc.compile()
    return nc


def _get_nc() -> bass.Bass:
    global _CACHED_NC
    if _CACHED_NC is None:
        _CACHED_NC = _build_nc()
    return _CACHED_NC


def _stripe_kxm(w: np.ndarray) -> np.ndarray:
    """[256, M] -> [128, 2, M] k-subtile layout (c = t*128 + p)."""
    return np.ascontiguousarray(w.reshape(2, 128, -1).transpose(1, 0, 2))


def make_in_maps(x, w_qkv, w_out):
    x2 = np.ascontiguousarray(x.reshape(B, C, N), dtype=np.float32)
    in_maps = []
    for core in range(8):
        b, h = divmod(core, HEADS)
        hs = slice(h * DIM_HEAD, (h + 1) * DIM_HEAD)
        wq = (w_qkv[0 * C :][hs, :] * SCALE).T  # [256, 64], scale folded
        wk = w_qkv[1 * C :][hs, :].T
        wv = w_qkv[2 * C :][hs, :].T
        wo = w_out[:, hs].T  # [64, 256]
        in_maps.append(
            {
                "x": x2[b],
                "wq": _stripe_kxm(wq.astype(np.float32)),
                "wk": _stripe_kxm(wk.astype(np.float32)),
                "wv": _stripe_kxm(wv.astype(np.float32)),
                "wo": np.ascontiguousarray(wo, dtype=np.float32),
            }
        )
    return in_maps


def combine(results, b_out):
    out = np.zeros((B, C, N), dtype=np.float32)
    for core in range(8):
        b, _h = divmod(core, HEADS)
        r = results[core]
        out[b] += r["u"].reshape(C, N) / r["dnm"].reshape(1, N)
    out += b_out.astype(np.float32)[None, :, None]
    return out.reshape(B, C, 64, 64)


def kernel(x, w_qkv, w_out, b_out, _run_kwargs=None):
    nc = _get_nc()
    in_maps = make_in_maps(np.asarray(x), np.asarray(w_qkv), np.asarray(w_out))
    kw = _run_kwargs or {}
    res = run_bass_kernel_spmd(nc, in_maps, list(range(8)), **kw)
    out = combine(res.results, np.asarray(b_out))
    kernel.last_result = res
    return out


# revision 11
# speedup vs baseline: 2.6506x; 1.0311x over previous
"""Trainium2 Bass kernel for nn_Attention (dense_transformer).

Sharding: 8 cores = 2 batches x 4 heads; each core computes one (batch, head)
attention independently (head/tensor parallel), QKV weights column-sharded and
the output projection row-sharded per head. Host sums the 4 per-head partial
output projections per batch (row-parallel unshard) and adds the bias.

Per-core dataflow:
  x_b [256, 4096] fp16 -> q = scale*Wq_h @ x, k = Wk_h @ x     (PE fp16, [64, 4096])
                          vT[m, d] = (x chunk)^T @ WvT_h        (PE fp16, [128, 64])
  T = k^T q   (scores^T, partition = key index m, K padded 128) (PE fp16 -> PSUM f32)
  E = exp(T)                                                    (ACT, PSUM -> SBUF fp16)
  [O; denom; 0pad] = [v; 1; 0]^T @ E, accumulated over m-blocks (PE fp16, [128, n])
  U = Wout_h @ O  (unnormalized)                                (PE f32r, [256, n])
Host: out_b = sum_h U_bh / denom_bh + b_out.

fp16 operands: ~2^-11 rounding; final absmax rel err ~4e-4 (vs 1.3e-6 all-fp32,
which runs 2.6x slower due to 2-pass fp32 matmuls + serial 4-byte weight loads).
"""

import numpy as np

import concourse.bass as bass
import concourse.tile as tile
from concourse import bacc, mybir
from concourse.bass_utils import run_bass_kernel_spmd

HEADS = 4
DIM_HEAD = 64
SCALE = DIM_HEAD**-0.5
B = 2
C = 256  # input channels
N = 4096  # spatial positions (64*64)
NCH = 1024  # n-chunk (query) size of the main pipeline
NB = N // 128  # number of 128-wide key blocks (32)
F32 = mybir.dt.float32
F32R = mybir.dt.float32r  # single-pass PE fp32 (vs 4 cyc/row for fp32)
F16 = mybir.dt.float16

_CACHED_NC = None


def _build_nc() -> bass.Bass:
    """Per-core program; identical on all 8 cores (SPMD), data differs."""
    nc = bacc.Bacc(None, target_bir_lowering=False, debug=False)

    x = nc.declare_dram_parameter("x", [C, N], F16, isOutput=False)
    wq = nc.declare_dram_parameter("wq", [128, 2, DIM_HEAD], F16, isOutput=False)
    wk = nc.declare_dram_parameter("wk", [128, 2, DIM_HEAD], F16, isOutput=False)
    wv = nc.declare_dram_parameter("wv", [128, 2, DIM_HEAD], F16, isOutput=False)
    wo = nc.declare_dram_parameter("wo", [DIM_HEAD, C], F32, isOutput=False)
    u = nc.declare_dram_parameter("u", [C, N], F32, isOutput=True)
    dnm = nc.declare_dram_parameter("dnm", [1, N], F32, isOutput=True)

    with (
        tile.TileContext(nc) as tc,
        tc.tile_pool(name="singles", bufs=1) as singles,
        tc.tile_pool(name="psum", bufs=2, space="PSUM") as psum,
        tc.tile_pool(name="psumO", bufs=1, space="PSUM") as psumO,
        tc.tile_pool(name="psumU", bufs=1, space="PSUM") as psumU,
        tc.tile_pool(name="esb", bufs=3) as esb,
        tc.tile_pool(name="osb", bufs=2) as osb,
        tc.tile_pool(name="usb", bufs=4) as usb,
    ):
        x0 = singles.tile([128, N], F16)  # channels 0..127
        x1 = singles.tile([128, N], F16)  # channels 128..255
        wq_sb = singles.tile([128, 2, DIM_HEAD], F16)
        wk_sb = singles.tile([128, 2, DIM_HEAD], F16)
        wv_sb = singles.tile([128, 2, DIM_HEAD], F16)
        wo_sb = singles.tile([DIM_HEAD, C], F32R)
        q_sb = singles.tile([128, N], F16)  # rows 64..127 zero (K padded to 128)
        k_sb = singles.tile([128, N], F16)
        # v'^T blocks: [m-block 128, d 64 | ones | zero-pad to 128] (M=128 -> FWL)
        vt_sb = singles.tile([128, NB, 128], F16)

        nc.sync.dma_start(wq_sb[:], wq[:])
        nc.sync.dma_start(wk_sb[:], wk[:])
        nc.sync.dma_start(wv_sb[:], wv[:])
        nc.sync.dma_start(wo_sb[:], wo[:].bitcast(F32R))
        # chunked x load so projections can start on early chunks
        XCH = N // 4
        for i in range(4):
            xsl = slice(i * XCH, (i + 1) * XCH)
            nc.sync.dma_start(x0[:, xsl], x[0:128, xsl])
            nc.sync.dma_start(x1[:, xsl], x[128:256, xsl])

        nc.vector.memset(q_sb[DIM_HEAD:128, :], 0.0)
        nc.vector.memset(k_sb[DIM_HEAD:128, :], 0.0)
        nc.vector.memset(vt_sb[:, :, DIM_HEAD:128], 0.0)
        ones_t = singles.tile([128, 1], F32)
        nc.vector.memset(ones_t[:], 1.0)
        nc.vector.tensor_copy(
            vt_sb[:, :, DIM_HEAD], ones_t[:, 0:1].to_broadcast((128, NB))
        )

        # ---- Phase B: projections (shared psum pool; interleaved emission) ----
        def proj_q(ch):
            sl = slice(ch * 512, (ch + 1) * 512)
            ps = psum.tile([DIM_HEAD, 512], F32, tag="t")
            nc.tensor.matmul(ps[:], wq_sb[:, 0, :], x0[:, sl], start=True, stop=False)
            nc.tensor.matmul(ps[:], wq_sb[:, 1, :], x1[:, sl], start=False, stop=True)
            nc.vector.tensor_copy(q_sb[0:DIM_HEAD, sl], ps[:])

        def proj_k(ch):
            sl = slice(ch * 512, (ch + 1) * 512)
            ps = psum.tile([DIM_HEAD, 512], F32, tag="t")
            nc.tensor.matmul(ps[:], wk_sb[:, 0, :], x0[:, sl], start=True, stop=False)
            nc.tensor.matmul(ps[:], wk_sb[:, 1, :], x1[:, sl], start=False, stop=True)
            nc.vector.tensor_copy(k_sb[0:DIM_HEAD, sl], ps[:])

        def proj_v(mb):
            sl = slice(mb * 128, (mb + 1) * 128)
            ps = psum.tile([128, DIM_HEAD], F32, tag="t")
            nc.tensor.matmul(ps[:], x0[:, sl], wv_sb[:, 0, :], start=True, stop=False)
            nc.tensor.matmul(ps[:], x1[:, sl], wv_sb[:, 1, :], start=False, stop=True)
            nc.vector.tensor_copy(vt_sb[:, mb, 0:DIM_HEAD], ps[:])

        # what ci=0 consumes first: q chunks 0-1, then (k chunk c, vt blocks 4c..)
        proj_q(0)
        proj_q(1)
        for ch in range(N // 512):
            proj_k(ch)
            for j in range(4):
                proj_v(ch * 4 + j)
            if ch >= 2:
                proj_q(ch)

        # ---- Phase C: attention + output projection, n-chunks of NCH ----
        for ci in range(N // NCH):
            n0 = ci * NCH
            ps_o = psumO.tile([128, NCH], F32)
            for mb in range(NB):
                msl = slice(mb * 128, (mb + 1) * 128)
                ps_t = psum.tile([128, NCH], F32, tag="t")
                e_t = esb.tile([128, NCH], F16)
                for s in range(NCH // 512):
                    ssl = slice(s * 512, (s + 1) * 512)
                    nc.tensor.matmul(
                        ps_t[:, ssl],
                        k_sb[:, msl],
                        q_sb[:, n0 + s * 512 : n0 + (s + 1) * 512],
                        start=True,
                        stop=True,
                    )
                nc.scalar.activation(e_t[:], ps_t[:], mybir.ActivationFunctionType.Exp)
                for s in range(NCH // 512):
                    ssl = slice(s * 512, (s + 1) * 512)
                    nc.tensor.matmul(
                        ps_o[:, ssl],
                        vt_sb[:, mb, :],
                        e_t[:, ssl],
                        start=(mb == 0),
                        stop=(mb == NB - 1),
                    )
            o_t = osb.tile([DIM_HEAD + 1, NCH], F32R)
            nc.vector.tensor_copy(o_t[:], ps_o[0 : DIM_HEAD + 1, :])
            nc.sync.dma_start(
                dnm[0:1, n0 : n0 + NCH], o_t[DIM_HEAD : DIM_HEAD + 1, :].bitcast(F32)
            )
            for half in range(2):
                osl = slice(half * 128, (half + 1) * 128)
                ps_u = psumU.tile([128, NCH], F32)
                for s in range(NCH // 512):
                    ssl = slice(s * 512, (s + 1) * 512)
                    nc.tensor.matmul(
                        ps_u[:, ssl],
                        wo_sb[:, osl],
                        o_t[0:DIM_HEAD, ssl],
                        start=True,
                        stop=True,
                    )
                u_t = usb.tile([128, NCH], F32)
                nc.vector.tensor_copy(u_t[:], ps_u[:])
                nc.sync.dma_start(u[osl, n0 : n0 + NCH], u_t[:])

    nc.compile()
    return nc


def _get_nc() -> bass.Bass:
    global _CACHED_NC
    if _CACHED_NC is None:
        _CACHED_NC = _build_nc()
    return _CACHED_NC


def _stripe_kxm(w: np.ndarray, dtype) -> np.ndarray:
    """[256, M] -> [128, 2, M] k-subtile layout (c = t*128 + p)."""
    return np.ascontiguousarray(w.reshape(2, 128, -1).transpose(1, 0, 2)).astype(dtype)


def make_in_maps(x, w_qkv, w_out):
    x2 = np.ascontiguousarray(x.reshape(B, C, N)).astype(np.float16)
    in_maps = []
    for core in range(8):
        b, h = divmod(core, HEADS)
        hs = slice(h * DIM_HEAD, (h + 1) * DIM_HEAD)
        wq_ = (w_qkv[0 * C :][hs, :] * SCALE).T  # [256, 64], scale folded
        wk_ = w_qkv[1 * C :][hs, :].T
        wv_ = w_qkv[2 * C :][hs, :].T
        wo_ = w_out[:, hs].T  # [64, 256]
        in_maps.append(
            {
                "x": x2[b],
                "wq": _stripe_kxm(wq_, np.float16),
                "wk": _stripe_kxm(wk_, np.float16),
                "wv": _stripe_kxm(wv_, np.float16),
                "wo": np.ascontiguousarray(wo_, dtype=np.float32),
            }
        )
    return in_maps


def combine(results, b_out):
    out = np.zeros((B, C, N), dtype=np.float32)
    for core in range(8):
        b, _h = divmod(core, HEADS)
        r = results[core]
        out[b] += r["u"].reshape(C, N) / r["dnm"].reshape(1, N)
    out += b_out.astype(np.float32)[None, :, None]
    return out.reshape(B, C, 64, 64)


def kernel(x, w_qkv, w_out, b_out, _run_kwargs=None):
    nc = _get_nc()
    in_maps = make_in_maps(np.asarray(x), np.asarray(w_qkv), np.asarray(w_out))
    kw = _run_kwargs or {}
    res = run_bass_kernel_spmd(nc, in_maps, list(range(8)), **kw)
    out = combine(res.results, np.asarray(b_out))
    kernel.last_result = res
    return out
